# revision 4
# baseline (speedup 1.0000x reference)
"""Discrete mixture (MoE-style routing) Bass kernel for Trainium2.

Reference computation (per batch row b):
    logits  = params[b, :K]
    gumbel  = -log(-log(uniform_noise[b]))
    sel     = argmax(logits + gumbel)                      # categorical sample
    comp    = params[b, K + sel*2D : K + (sel+1)*2D]       # gather routed expert params
    mean, log_std = comp[:D], comp[D:]
    out[b]  = mean + exp(log_std) * eps[b]

Sharding: pure data parallel over the batch axis across 8 NeuronCores
(128 rows per core). Each core reads only its logits (32KB), noise (32KB),
eps (1MB), the *gathered* 2MB of routed component params (indirect DMA),
and writes 1MB — ~4MB of HBM traffic per core instead of the 134MB full
params shard.
"""

import numpy as np

import concourse.bacc as bacc
import concourse.bass as bass
import concourse.tile as tile
from concourse import mybir
from concourse.bass_utils import run_bass_kernel_spmd

AF = mybir.ActivationFunctionType
ALU = mybir.AluOpType

B = 1024
K = 64
D = 2048
TWO_D = 2 * D
TOTAL = K + K * TWO_D  # 262208
N_CORES = 8
ROWS = B // N_CORES  # 128 rows per core == SBUF partition count

_CACHE: dict = {}


def _build_program() -> bass.Bass:
    nc = bacc.Bacc("TRN2", target_bir_lowering=False, debug=False)

    params = nc.dram_tensor(
        "params", [ROWS, TOTAL], mybir.dt.float32, kind="ExternalInput"
    ).ap()
    noise = nc.dram_tensor(
        "uniform_noise", [ROWS, K], mybir.dt.float32, kind="ExternalInput"
    ).ap()
    eps = nc.dram_tensor(
        "eps", [ROWS, D], mybir.dt.float32, kind="ExternalInput"
    ).ap()
    rowbase = nc.dram_tensor(
        "rowbase", [ROWS, 1], mybir.dt.uint32, kind="ExternalInput"
    ).ap()
    out = nc.dram_tensor(
        "out", [ROWS, D], mybir.dt.float32, kind="ExternalOutput"
    ).ap()

    with tile.TileContext(nc) as tc:
        with tc.tile_pool(name="p", bufs=1) as pool:
            logits_t = pool.tile([ROWS, K], mybir.dt.float32)
            noise_t = pool.tile([ROWS, K], mybir.dt.float32)
            rowbase_t = pool.tile([ROWS, 1], mybir.dt.uint32)
            eps_t = pool.tile([ROWS, D], mybir.dt.float32)

            nc.sync.dma_start(out=logits_t[:], in_=params[:, 0:K])
            nc.sync.dma_start(out=noise_t[:], in_=noise[:])
            nc.sync.dma_start(out=rowbase_t[:], in_=rowbase[:])
            nc.sync.dma_start(out=eps_t[:], in_=eps[:])

            # scores = logits - log(-log(u))  (== logits + gumbel)
            t1 = pool.tile([ROWS, K], mybir.dt.float32)
            nc.scalar.activation(t1[:], noise_t[:], AF.Ln)
            nc.scalar.activation(t1[:], t1[:], AF.Ln, scale=-1.0)
            scores = pool.tile([ROWS, K], mybir.dt.float32)
            nc.vector.tensor_tensor(
                out=scores[:], in0=logits_t[:], in1=t1[:], op=ALU.subtract
            )

            # per-row argmax
            max8 = pool.tile([ROWS, 8], mybir.dt.float32)
            idx8 = pool.tile([ROWS, 8], mybir.dt.uint32)
            nc.vector.max_with_indices(max8[:], idx8[:], scores[:])

            # flat element offsets into params: rowbase[p] + sel[p]*2D
            # (rowbase[p] = p*TOTAL + K, supplied as a host constant)
            offs = pool.tile([ROWS, 1], mybir.dt.uint32)
            nc.vector.tensor_scalar(
                out=offs[:],
                in0=idx8[:, 0:1],
                scalar1=TWO_D,
                scalar2=None,
                op0=ALU.mult,
            )
            nc.vector.tensor_tensor(
                out=offs[:], in0=offs[:], in1=rowbase_t[:], op=ALU.add
            )

            # gather the selected component's [2D] param chunk per row
            comp = pool.tile([ROWS, TWO_D], mybir.dt.float32)
            nc.gpsimd.indirect_dma_start(
                out=comp[:],
                out_offset=None,
                in_=params[:, :],
                in_offset=bass.IndirectOffsetOnAxis(ap=offs[:, 0:1], axis=1),
            )

            # out = mean + exp(log_std) * eps
            std = pool.tile([ROWS, D], mybir.dt.float32)
            nc.scalar.activation(std[:], comp[:, D:TWO_D], AF.Exp)
            res = pool.tile([ROWS, D], mybir.dt.float32)
            nc.vector.tensor_tensor(
                out=res[:], in0=std[:], in1=eps_t[:], op=ALU.mult
            )
            nc.vector.tensor_tensor(
                out=res[:], in0=res[:], in1=comp[:, 0:D], op=ALU.add
            )
            nc.sync.dma_start(out=out[:], in_=res[:])

    nc.finalize()
    return nc


def _get_program() -> bass.Bass:
    if "nc" not in _CACHE:
        _CACHE["nc"] = _build_program()
    return _CACHE["nc"]


def make_in_maps(params, uniform_noise, eps):
    params = np.ascontiguousarray(params, dtype=np.float32)
    uniform_noise = np.ascontiguousarray(uniform_noise, dtype=np.float32)
    eps = np.ascontiguousarray(eps, dtype=np.float32)
    rowbase = (
        np.arange(ROWS, dtype=np.uint64) * TOTAL + K
    ).astype(np.uint32).reshape(ROWS, 1)
    in_maps = []
    for i in range(N_CORES):
        sl = slice(i * ROWS, (i + 1) * ROWS)
        in_maps.append(
            {
                "params": params[sl],
                "uniform_noise": uniform_noise[sl],
                "eps": eps[sl],
                "rowbase": rowbase,
            }
        )
    return in_maps


def kernel(params, uniform_noise, eps, **run_kwargs):
    nc = _get_program()
    in_maps = make_in_maps(params, uniform_noise, eps)
    res = run_bass_kernel_spmd(nc, in_maps, list(range(N_CORES)), **run_kwargs)
    out = np.concatenate([r["out"] for r in res.results], axis=0)
    if run_kwargs:
        _CACHE["last_results"] = res
    return out


# revision 7
# speedup vs baseline: 1.1012x; 1.1012x over previous
"""Discrete mixture (MoE-style routing) Bass kernel for Trainium2.

Reference computation (per batch row b):
    logits  = params[b, :K]
    gumbel  = -log(-log(uniform_noise[b]))
    sel     = argmax(logits + gumbel)                      # categorical sample
    comp    = params[b, K + sel*2D : K + (sel+1)*2D]       # gather routed expert params
    mean, log_std = comp[:D], comp[D:]
    out[b]  = mean + exp(log_std) * eps[b]

Sharding: pure data parallel over the batch axis across 8 NeuronCores
(128 rows per core, one row per SBUF partition). Each core reads only its
routing metadata (33KB aux), eps (1MB), and the *gathered* 2MB of routed
component params via indirect DMA — ~4MB of HBM traffic per core instead
of the 134MB full params shard.

Layout tricks:
  - logits / uniform_noise / per-row gather base offsets are packed into a
    single [128, 129] uint32 "aux" tensor on the host so the routing
    prologue needs one small DMA dispatch instead of three.
  - the gather is split into log_std-first + mean indirect DMAs (same
    row offsets, element_offset selects the half), so exp/mult overlap the
    mean gather; compute and store are column-chunked to pipeline.
"""

import numpy as np

import concourse.bacc as bacc
import concourse.bass as bass
import concourse.tile as tile
from concourse import mybir
from concourse.bass_utils import run_bass_kernel_spmd

AF = mybir.ActivationFunctionType
ALU = mybir.AluOpType

B = 1024
K = 64
D = 2048
TWO_D = 2 * D
TOTAL = K + K * TWO_D  # 262208
N_CORES = 8
ROWS = B // N_CORES  # 128 rows per core == SBUF partition count
N_CHUNK = 2
DC = D // N_CHUNK

_CACHE: dict = {}


def _build_program() -> bass.Bass:
    nc = bacc.Bacc("TRN2", target_bir_lowering=False, debug=False)

    params = nc.dram_tensor(
        "params", [ROWS, TOTAL], mybir.dt.float32, kind="ExternalInput"
    ).ap()
    aux = nc.dram_tensor(
        "aux", [ROWS, 2 * K + 1], mybir.dt.uint32, kind="ExternalInput"
    ).ap()
    eps = nc.dram_tensor(
        "eps", [ROWS, D], mybir.dt.float32, kind="ExternalInput"
    ).ap()
    out = nc.dram_tensor(
        "out", [ROWS, D], mybir.dt.float32, kind="ExternalOutput"
    ).ap()

    with tile.TileContext(nc) as tc:
        with tc.tile_pool(name="p", bufs=1) as pool:
            aux_t = pool.tile([ROWS, 2 * K + 1], mybir.dt.uint32)
            eps_t = pool.tile([ROWS, D], mybir.dt.float32)
            nc.sync.dma_start(out=aux_t[:], in_=aux[:])
            nc.sync.dma_start(out=eps_t[:], in_=eps[:])

            logits_v = aux_t[:, 0:K].bitcast(mybir.dt.float32)
            noise_v = aux_t[:, K : 2 * K].bitcast(mybir.dt.float32)
            rowbase_v = aux_t[:, 2 * K : 2 * K + 1]

            # scores = logits - log(-log(u))  (== logits + gumbel)
            t1 = pool.tile([ROWS, K], mybir.dt.float32)
            nc.scalar.activation(t1[:], noise_v, AF.Ln)
            nc.scalar.activation(t1[:], t1[:], AF.Ln, scale=-1.0)
            scores = pool.tile([ROWS, K], mybir.dt.float32)
            nc.vector.tensor_tensor(
                out=scores[:], in0=logits_v, in1=t1[:], op=ALU.subtract
            )

            # per-row argmax
            max8 = pool.tile([ROWS, 8], mybir.dt.float32)
            idx8 = pool.tile([ROWS, 8], mybir.dt.uint32)
            nc.vector.max_with_indices(max8[:], idx8[:], scores[:])

            # flat element offsets into params: rowbase[p] + sel[p]*2D
            # (rowbase[p] = p*TOTAL + K, packed into aux on the host)
            offs = pool.tile([ROWS, 1], mybir.dt.uint32)
            nc.vector.tensor_scalar(
                out=offs[:],
                in0=idx8[:, 0:1],
                scalar1=TWO_D,
                scalar2=None,
                op0=ALU.mult,
            )
            nc.vector.tensor_tensor(
                out=offs[:], in0=offs[:], in1=rowbase_v, op=ALU.add
            )
            offs_ls = pool.tile([ROWS, 1], mybir.dt.uint32)
            nc.vector.tensor_scalar(
                out=offs_ls[:],
                in0=offs[:],
                scalar1=D,
                scalar2=None,
                op0=ALU.add,
            )

            # gather the selected component's params: log_std half first
            # (feeds exp), mean half second (needed only for the final add).
            ls_t = pool.tile([ROWS, D], mybir.dt.float32)
            mean_t = pool.tile([ROWS, D], mybir.dt.float32)
            nc.gpsimd.indirect_dma_start(
                out=ls_t[:],
                out_offset=None,
                in_=params[:, :],
                in_offset=bass.IndirectOffsetOnAxis(ap=offs_ls[:, 0:1], axis=1),
            )
            nc.gpsimd.indirect_dma_start(
                out=mean_t[:],
                out_offset=None,
                in_=params[:, :],
                in_offset=bass.IndirectOffsetOnAxis(ap=offs[:, 0:1], axis=1),
            )

            # out = mean + exp(log_std) * eps, column-chunked to pipeline
            # ACT (exp) / DVE (mult, add) / store behind the gathers.
            std = pool.tile([ROWS, D], mybir.dt.float32)
            res = pool.tile([ROWS, D], mybir.dt.float32)
            for c in range(N_CHUNK):
                sl = slice(c * DC, (c + 1) * DC)
                nc.scalar.activation(std[:, sl], ls_t[:, sl], AF.Exp)
                nc.vector.tensor_tensor(
                    out=res[:, sl], in0=std[:, sl], in1=eps_t[:, sl], op=ALU.mult
                )
                nc.vector.tensor_tensor(
                    out=res[:, sl], in0=res[:, sl], in1=mean_t[:, sl], op=ALU.add
                )
                nc.sync.dma_start(out=out[:, sl], in_=res[:, sl])

    nc.finalize()
    return nc


def _get_program() -> bass.Bass:
    if "nc" not in _CACHE:
        _CACHE["nc"] = _build_program()
    return _CACHE["nc"]


def make_in_maps(params, uniform_noise, eps):
    params = np.ascontiguousarray(params, dtype=np.float32)
    uniform_noise = np.ascontiguousarray(uniform_noise, dtype=np.float32)
    eps = np.ascontiguousarray(eps, dtype=np.float32)
    rowbase = (np.arange(ROWS, dtype=np.uint64) * TOTAL + K).astype(np.uint32)
    in_maps = []
    for i in range(N_CORES):
        sl = slice(i * ROWS, (i + 1) * ROWS)
        aux = np.empty((ROWS, 2 * K + 1), np.uint32)
        aux[:, 0:K] = np.ascontiguousarray(params[sl, :K]).view(np.uint32)
        aux[:, K : 2 * K] = uniform_noise[sl].view(np.uint32)
        aux[:, 2 * K] = rowbase
        in_maps.append(
            {
                "params": params[sl],
                "aux": aux,
                "eps": eps[sl],
            }
        )
    return in_maps


def kernel(params, uniform_noise, eps, **run_kwargs):
    nc = _get_program()
    in_maps = make_in_maps(params, uniform_noise, eps)
    res = run_bass_kernel_spmd(nc, in_maps, list(range(N_CORES)), **run_kwargs)
    out = np.concatenate([r["out"] for r in res.results], axis=0)
    if run_kwargs:
        _CACHE["last_results"] = res
    return out


# revision 11
# speedup vs baseline: 1.2433x; 1.1290x over previous
"""Discrete mixture (MoE-style routing) Bass kernel for Trainium2.

Reference computation (per batch row b):
    logits  = params[b, :K]
    gumbel  = -log(-log(uniform_noise[b]))
    sel     = argmax(logits + gumbel)                      # categorical sample
    comp    = params[b, K + sel*2D : K + (sel+1)*2D]       # gather routed expert params
    mean, log_std = comp[:D], comp[D:]
    out[b]  = mean + exp(log_std) * eps[b]

Sharding: pure data parallel over the batch axis across 8 NeuronCores
(128 rows per core, one row per SBUF partition). Each core reads only its
routing metadata (one small aux DMA), eps (1MB), and the *gathered* 2MB of
routed component params via indirect DMA — ~4MB of HBM traffic per core
instead of the 134MB full params shard.

Pipelining: the gather is split into column chunks ordered
ls0, mean0, ls1, mean1 (log_std first — it feeds exp), each with its own
per-row offset vector (sel*2D + per-chunk base, bases packed into aux), so
exp/mult/add/store stream behind the gather chunks.
"""

import numpy as np

import concourse.bacc as bacc
import concourse.bass as bass
import concourse.tile as tile
from concourse import mybir
from concourse.bass_utils import run_bass_kernel_spmd

AF = mybir.ActivationFunctionType
ALU = mybir.AluOpType

B = 1024
K = 64
D = 2048
TWO_D = 2 * D
TOTAL = K + K * TWO_D  # 262208
N_CORES = 8
ROWS = B // N_CORES  # 128 rows per core == SBUF partition count
N_CHUNK = 2
DC = D // N_CHUNK  # 1024
AUX_W = 2 * K + 2 * N_CHUNK  # logits | noise | ls bases | mean bases

_CACHE: dict = {}


def _build_program() -> bass.Bass:
    nc = bacc.Bacc("TRN2", target_bir_lowering=False, debug=False)

    params = nc.dram_tensor(
        "params", [ROWS, TOTAL], mybir.dt.float32, kind="ExternalInput"
    ).ap()
    aux = nc.dram_tensor(
        "aux", [ROWS, AUX_W], mybir.dt.uint32, kind="ExternalInput"
    ).ap()
    eps = nc.dram_tensor(
        "eps", [ROWS, D], mybir.dt.float32, kind="ExternalInput"
    ).ap()
    out = nc.dram_tensor(
        "out", [ROWS, D], mybir.dt.float32, kind="ExternalOutput"
    ).ap()

    with tile.TileContext(nc) as tc:
        with tc.tile_pool(name="p", bufs=1) as pool:
            aux_t = pool.tile([ROWS, AUX_W], mybir.dt.uint32)
            eps_t = pool.tile([ROWS, D], mybir.dt.float32)
            nc.sync.dma_start(out=aux_t[:], in_=aux[:])
            nc.sync.dma_start(out=eps_t[:], in_=eps[:])

            logits_v = aux_t[:, 0:K].bitcast(mybir.dt.float32)
            noise_v = aux_t[:, K : 2 * K].bitcast(mybir.dt.float32)
            # per-chunk gather bases: [ls0, ls1, ..., mean0, mean1, ...]
            base_ls = [aux_t[:, 2 * K + c : 2 * K + c + 1] for c in range(N_CHUNK)]
            base_mean = [
                aux_t[:, 2 * K + N_CHUNK + c : 2 * K + N_CHUNK + c + 1]
                for c in range(N_CHUNK)
            ]

            # scores = logits - log(-log(u))  (== logits + gumbel)
            t1 = pool.tile([ROWS, K], mybir.dt.float32)
            nc.scalar.activation(t1[:], noise_v, AF.Ln)
            nc.scalar.activation(t1[:], t1[:], AF.Ln, scale=-1.0)
            scores = pool.tile([ROWS, K], mybir.dt.float32)
            nc.vector.tensor_tensor(
                out=scores[:], in0=logits_v, in1=t1[:], op=ALU.subtract
            )

            # per-row argmax
            max8 = pool.tile([ROWS, 8], mybir.dt.float32)
            idx8 = pool.tile([ROWS, 8], mybir.dt.uint32)
            nc.vector.max_with_indices(max8[:], idx8[:], scores[:])

            sel = idx8[:, 0:1]
            ls_t = pool.tile([ROWS, D], mybir.dt.float32)
            mean_t = pool.tile([ROWS, D], mybir.dt.float32)
            std = pool.tile([ROWS, D], mybir.dt.float32)
            res = pool.tile([ROWS, D], mybir.dt.float32)
            offs_ls = [
                pool.tile([ROWS, 1], mybir.dt.uint32, name=f"offs_ls{c}")
                for c in range(N_CHUNK)
            ]
            offs_mean = [
                pool.tile([ROWS, 1], mybir.dt.uint32, name=f"offs_mean{c}")
                for c in range(N_CHUNK)
            ]

            def gather(dst_slice, offs_tile):
                nc.gpsimd.indirect_dma_start(
                    out=dst_slice,
                    out_offset=None,
                    in_=params[:, :],
                    in_offset=bass.IndirectOffsetOnAxis(
                        ap=offs_tile[:, 0:1], axis=1
                    ),
                )

            # offsets + gathers, log_std chunks first (they feed exp).
            sel4 = pool.tile([ROWS, 1], mybir.dt.uint32)
            nc.vector.tensor_scalar(
                out=sel4[:], in0=sel, scalar1=TWO_D, scalar2=None, op0=ALU.mult
            )
            for c in range(N_CHUNK):
                sl = slice(c * DC, (c + 1) * DC)
                nc.vector.tensor_tensor(
                    out=offs_ls[c][:], in0=sel4[:], in1=base_ls[c], op=ALU.add
                )
                gather(ls_t[:, sl], offs_ls[c])
                nc.vector.tensor_tensor(
                    out=offs_mean[c][:], in0=sel4[:], in1=base_mean[c], op=ALU.add
                )
                gather(mean_t[:, sl], offs_mean[c])

            # out = mean + exp(log_std) * eps, column-chunked to stream
            # ACT (exp) / DVE (mult, add) / store behind the gather chunks.
            for c in range(N_CHUNK):
                sl = slice(c * DC, (c + 1) * DC)
                nc.scalar.activation(std[:, sl], ls_t[:, sl], AF.Exp)
                nc.vector.tensor_tensor(
                    out=res[:, sl], in0=std[:, sl], in1=eps_t[:, sl], op=ALU.mult
                )
                nc.vector.tensor_tensor(
                    out=res[:, sl], in0=res[:, sl], in1=mean_t[:, sl], op=ALU.add
                )
                nc.sync.dma_start(out=out[:, sl], in_=res[:, sl])

    nc.finalize()
    return nc


def _get_program() -> bass.Bass:
    if "nc" not in _CACHE:
        _CACHE["nc"] = _build_program()
    return _CACHE["nc"]


def make_in_maps(params, uniform_noise, eps):
    params = np.ascontiguousarray(params, dtype=np.float32)
    uniform_noise = np.ascontiguousarray(uniform_noise, dtype=np.float32)
    eps = np.ascontiguousarray(eps, dtype=np.float32)
    row = np.arange(ROWS, dtype=np.uint64) * TOTAL + K
    in_maps = []
    for i in range(N_CORES):
        sl = slice(i * ROWS, (i + 1) * ROWS)
        aux = np.empty((ROWS, AUX_W), np.uint32)
        aux[:, 0:K] = np.ascontiguousarray(params[sl, :K]).view(np.uint32)
        aux[:, K : 2 * K] = uniform_noise[sl].view(np.uint32)
        for c in range(N_CHUNK):
            aux[:, 2 * K + c] = (row + D + c * DC).astype(np.uint32)
            aux[:, 2 * K + N_CHUNK + c] = (row + c * DC).astype(np.uint32)
        in_maps.append(
            {
                "params": params[sl],
                "aux": aux,
                "eps": eps[sl],
            }
        )
    return in_maps


def kernel(params, uniform_noise, eps, **run_kwargs):
    nc = _get_program()
    in_maps = make_in_maps(params, uniform_noise, eps)
    res = run_bass_kernel_spmd(nc, in_maps, list(range(N_CORES)), **run_kwargs)
    out = np.concatenate([r["out"] for r in res.results], axis=0)
    if run_kwargs:
        _CACHE["last_results"] = res
    return out
